# revision 1
# baseline (speedup 1.0000x reference)
"""LocallyConnected1d Trainium2 kernel.

Problem: out[b, oc, w] = sum_{ic,k} xp[b, ic, w+k] * W[w, oc, ic, k] + bias[oc, w]
  x: (32, 64, 2048) f32, weights: (2048, 64, 64, 3) f32, bias: (64, 2048) f32
  out: (32, 64, 2048) f32.  xp = x padded by 1 on both sides of the last axis.

Sharding: output_width (2048) is split into 8 contiguous chunks of 256, one per
NeuronCore.  Weights dominate the traffic (100 MB) and are fully sharded this
way (12.6 MB/core); x is sent with a 2-column halo.

Per-core compute: for each position w the contraction over (ic, k) + bias is a
193-term dot product, done as two PSUM-accumulated fp32 matmuls:
  mm1: K=128 rows = (k=0, ic=0..63) ++ (k=1, ic=0..63),  lhsT=[128, 64oc], rhs=[128, 32b]
  mm2: K=65  rows = (k=2, ic=0..63) ++ bias row,         lhsT=[65, 64oc],  rhs=[65, 32b]
The bias is folded in as lhsT row 64 of mm2 against a constant ones row in rhs.

fp32 matmuls lower to 2 HW passes (LDW+MM each); with N=32 the MM pass costs
N*4 = 128 PE cycles, so the PE floor is ~512 cyc/position at the observed
1.2 GHz clock (~110 us/core).  DMA (21 MB/core) is packet-rate-bound, so
weights/x are fetched in fat 64-position slices (4-16 KB contiguous per
partition) while PSUM/compute runs in 16-position chunks (1 bank each).

Host-side prep (numpy, cheap vs. the 100MB HBM traffic on device):
  wa[j, w, oc] = W[ws+w, oc, j%64, j//64]        j in [0,128)   (k-major)
  wb[j, w, oc] = W[ws+w, oc, j, 2] for j<64;  wb[64, w, oc] = bias[oc, ws+w]
  x1[j, c, b]  = xp[b, j%64, ws+c + j//64]       j in [0,128)
  x2[j, c, b]  = xp[b, j, ws+c+2] for j<64;   x2[64, c, b] = 1.0
"""

import numpy as np

import concourse.bacc as bacc
import concourse.mybir as mybir
import concourse.tile as tile
from concourse.bass_utils import run_bass_kernel_spmd

B, IC, OC, KS, W = 32, 64, 64, 3, 2048
NCORES = 8
OWC = W // NCORES  # 256 positions per core
CH = 16            # compute chunk; psum tile = [64, CH*32] = one bank
DCH = 64           # DMA chunk (positions per weight/x fetch)
DT = mybir.dt.float32

_compiled_nc = None


def _build_nc():
    nc = bacc.Bacc("TRN2")

    x1_d = nc.dram_tensor("x1", [2 * IC, OWC, B], DT, kind="ExternalInput")
    x2_d = nc.dram_tensor("x2", [IC + 1, OWC, B], DT, kind="ExternalInput")
    wa_d = nc.dram_tensor("wa", [2 * IC, OWC, OC], DT, kind="ExternalInput")
    wb_d = nc.dram_tensor("wb", [IC + 1, OWC, OC], DT, kind="ExternalInput")
    out_d = nc.dram_tensor("out", [OC, OWC, B], DT, kind="ExternalOutput")

    # First DMA slice is small so the PE starts quickly; the rest are fat.
    dma_slices = [(0, CH), (CH, DCH - CH)]
    p = DCH
    while p < OWC:
        dma_slices.append((p, min(DCH, OWC - p)))
        p += DCH

    with tile.TileContext(nc) as tc:
        with (
            tc.tile_pool(name="w", bufs=2) as wpool,
            tc.tile_pool(name="x", bufs=2) as xpool,
            tc.tile_pool(name="o", bufs=3) as opool,
            tc.tile_pool(name="ps", bufs=4, space="PSUM") as pspool,
        ):
            loaded = []  # (start, len, wa, wb, x1, x2)

            def load_slice(si):
                p0, plen = dma_slices[si]
                sl = slice(p0, p0 + plen)
                wa = wpool.tile([2 * IC, plen, OC], DT, tag="wa", name=f"wa_{si}")
                wb = wpool.tile([IC + 1, plen, OC], DT, tag="wb", name=f"wb_{si}")
                x1 = xpool.tile([2 * IC, plen, B], DT, tag="x1", name=f"x1_{si}")
                x2 = xpool.tile([IC + 1, plen, B], DT, tag="x2", name=f"x2_{si}")
                # slice 0 gates the PE start: split its loads across the two
                # HWDGE queues (sync + scalar) so descriptor issue overlaps.
                eng2 = nc.scalar if si == 0 else nc.sync
                nc.sync.dma_start(out=wa[:], in_=wa_d[:, sl, :])
                nc.sync.dma_start(out=x1[:], in_=x1_d[:, sl, :])
                eng2.dma_start(out=wb[:], in_=wb_d[:, sl, :])
                eng2.dma_start(out=x2[:], in_=x2_d[:, sl, :])
                loaded.append((p0, plen, wa, wb, x1, x2))

            # Software-pipelined emission: loads for slice si+1 are emitted
            # just before slice si's compute, so the HWDGE queue never holds
            # more than ~1 slice of prefetch during the ramp and the critical
            # early slices get the DMA engines to themselves.
            load_slice(0)
            load_slice(1)
            for si in range(len(dma_slices)):
                if si >= 1 and si + 1 < len(dma_slices):
                    load_slice(si + 1)
                p0, plen, wa, wb, x1, x2 = loaded[si]
                for c0 in range(0, plen, CH):
                    cl = min(CH, plen - c0)
                    ps = pspool.tile([OC, cl, B], DT, tag="ps", name=f"ps_{p0 + c0}")
                    for w in range(cl):
                        wl = c0 + w
                        nc.tensor.matmul(
                            ps[:, w, :],
                            wa[:, wl, :],
                            x1[:, wl, :],
                            start=True,
                            stop=False,
                        )
                        nc.tensor.matmul(
                            ps[:, w, :],
                            wb[:, wl, :],
                            x2[:, wl, :],
                            start=False,
                            stop=True,
                        )
                    ob = opool.tile([OC, cl, B], DT, tag="ob", name=f"ob_{p0 + c0}")
                    nc.scalar.copy(out=ob[:], in_=ps[:])
                    nc.sync.dma_start(
                        out=out_d[:, p0 + c0 : p0 + c0 + cl, :], in_=ob[:]
                    )

    nc.compile()
    return nc


def _get_nc():
    global _compiled_nc
    if _compiled_nc is None:
        _compiled_nc = _build_nc()
    return _compiled_nc


def shard_inputs(x, weights, bias):
    x = np.ascontiguousarray(np.asarray(x, dtype=np.float32))
    weights = np.asarray(weights, dtype=np.float32)
    bias = np.asarray(bias, dtype=np.float32)

    xp = np.pad(x, ((0, 0), (0, 0), (1, 1)))
    xpT = np.ascontiguousarray(xp.transpose(1, 2, 0))  # (IC, W+2, B)
    ones = np.ones((1, OWC, B), np.float32)

    in_maps = []
    for c in range(NCORES):
        ws = c * OWC
        x1 = np.concatenate(
            [xpT[:, ws : ws + OWC, :], xpT[:, ws + 1 : ws + 1 + OWC, :]], axis=0
        )
        x2 = np.concatenate([xpT[:, ws + 2 : ws + 2 + OWC, :], ones], axis=0)
        wsl = weights[ws : ws + OWC]  # (OWC, OC, IC, KS)
        wa = np.ascontiguousarray(wsl[:, :, :, 0:2].transpose(3, 2, 0, 1)).reshape(
            2 * IC, OWC, OC
        )
        wb = np.concatenate(
            [wsl[:, :, :, 2].transpose(2, 0, 1), bias[:, ws : ws + OWC].T[None]],
            axis=0,
        )
        in_maps.append(
            {
                "x1": np.ascontiguousarray(x1),
                "x2": np.ascontiguousarray(x2),
                "wa": np.ascontiguousarray(wa),
                "wb": np.ascontiguousarray(wb),
            }
        )
    return in_maps


def run_sharded(x, weights, bias, trace=False):
    nc = _get_nc()
    in_maps = shard_inputs(x, weights, bias)
    res = run_bass_kernel_spmd(nc, in_maps, list(range(NCORES)), trace=trace)
    out = np.empty((B, OC, W), np.float32)
    for c in range(NCORES):
        out[:, :, c * OWC : (c + 1) * OWC] = res.results[c]["out"].transpose(2, 0, 1)
    return out, res


def kernel(x, weights, bias):
    out, _ = run_sharded(x, weights, bias)
    return out



# revision 2
# speedup vs baseline: 2.2118x; 2.2118x over previous
"""LocallyConnected1d Trainium2 kernel.

Problem: out[b, oc, w] = sum_{ic,k} xp[b, ic, w+k] * W[w, oc, ic, k] + bias[oc, w]
  x: (32, 64, 2048) f32, weights: (2048, 64, 64, 3) f32, bias: (64, 2048) f32
  out: (32, 64, 2048) f32.  xp = x padded by 1 on both sides of the last axis.

Sharding: output_width (2048) split into 8 contiguous chunks of 256, one per
NeuronCore.  Weights dominate traffic and are fully sharded this way.

All device data is bf16 (tolerance is 2e-2; bf16 end-to-end error ~3e-3).
Per position w the contraction over the 193 terms (ic*k + bias) is done as two
PSUM-accumulated matmuls with the X PATCH as the stationary operand (lhsT) and
the WEIGHTS as the moving operand (rhs):
  mm1: K=128 rows = (k=0, ic=0..63) ++ (k=1, ic=0..63), lhsT=[128,32b], rhs=[128,64oc]
  mm2: K=65  rows = (k=2, ic=0..63) ++ ones row,        lhsT=[65,32b],  rhs=[65,64oc]
The bias is rhs row 64 of mm2 against the constant ones row in lhsT.

Why flipped vs the obvious orientation: LDWEIGHTS cost scales with the lhsT
free size (columns loaded into the PE array), so loading the 32-wide x patch
(27ns) instead of the 64-wide fp32 weight (107ns) removes the LDW bottleneck
that dominated the fp32 version (1024 LDW x 107ns = 110us).

Host-side prep (numpy -> bf16):
  xb[j, c, b] = xp[b, j%64, ws+c + j//64]   j in [0,128)   (k-major)
  x2[j, c, b] = xp[b, j, ws+c+2] for j<64;  x2[64, c, b] = 1.0
  wa[j, c, oc] = W[ws+c, oc, j%64, j//64]   j in [0,128)
  wb[j, c, oc] = W[ws+c, oc, j, 2] for j<64; wb[64, c, oc] = bias[oc, ws+c]
Output out_d[b, c, oc] (bf16) -> host transpose to (B, OC, W) fp32.
"""

import numpy as np
import ml_dtypes

import concourse.bacc as bacc
import concourse.mybir as mybir
import concourse.tile as tile
from concourse.bass_utils import run_bass_kernel_spmd

B, IC, OC, KS, W = 32, 64, 64, 3, 2048
NCORES = 8
OWC = W // NCORES  # 256 positions per core
CH = 8             # positions per psum tile: [32, 8, 64] = 512 f32/part = 1 bank
DCH = 64           # DMA chunk (positions per weight/x fetch)
DT = mybir.dt.bfloat16
BF16 = ml_dtypes.bfloat16

_compiled_nc = None


def _build_nc():
    nc = bacc.Bacc("TRN2")

    xb_d = nc.dram_tensor("xb", [2 * IC, OWC, B], DT, kind="ExternalInput")
    x2_d = nc.dram_tensor("x2", [IC + 1, OWC, B], DT, kind="ExternalInput")
    wa_d = nc.dram_tensor("wa", [2 * IC, OWC, OC], DT, kind="ExternalInput")
    wb_d = nc.dram_tensor("wb", [IC + 1, OWC, OC], DT, kind="ExternalInput")
    out_d = nc.dram_tensor("out", [B, OWC, OC], DT, kind="ExternalOutput")

    # First DMA slice is small so the PE starts quickly; the rest are fat.
    dma_slices = [(0, CH), (CH, DCH - CH)]
    p = DCH
    while p < OWC:
        dma_slices.append((p, min(DCH, OWC - p)))
        p += DCH

    with tile.TileContext(nc) as tc:
        with (
            tc.tile_pool(name="w", bufs=2) as wpool,
            tc.tile_pool(name="x", bufs=2) as xpool,
            tc.tile_pool(name="o", bufs=3) as opool,
            tc.tile_pool(name="ps", bufs=6, space="PSUM") as pspool,
        ):
            loaded = []  # (start, len, wa, wb, xb, x2)

            def load_slice(si):
                p0, plen = dma_slices[si]
                sl = slice(p0, p0 + plen)
                wa = wpool.tile([2 * IC, plen, OC], DT, tag="wa", name=f"wa_{si}")
                wb = wpool.tile([IC + 1, plen, OC], DT, tag="wb", name=f"wb_{si}")
                xb = xpool.tile([2 * IC, plen, B], DT, tag="xb", name=f"xb_{si}")
                x2 = xpool.tile([IC + 1, plen, B], DT, tag="x2", name=f"x2_{si}")
                eng2 = nc.scalar if si == 0 else nc.sync
                nc.sync.dma_start(out=wa[:], in_=wa_d[:, sl, :])
                nc.sync.dma_start(out=xb[:], in_=xb_d[:, sl, :])
                eng2.dma_start(out=wb[:], in_=wb_d[:, sl, :])
                eng2.dma_start(out=x2[:], in_=x2_d[:, sl, :])
                loaded.append((p0, plen, wa, wb, xb, x2))

            load_slice(0)
            load_slice(1)
            ncopy = 0
            for si in range(len(dma_slices)):
                if si >= 1 and si + 1 < len(dma_slices):
                    load_slice(si + 1)
                p0, plen, wa, wb, xb, x2 = loaded[si]
                ob = opool.tile([B, plen, OC], DT, tag="ob", name=f"ob_{si}")
                for c0 in range(0, plen, CH):
                    cl = min(CH, plen - c0)
                    ps = pspool.tile([B, cl, OC], mybir.dt.float32, tag="ps",
                                     name=f"ps_{p0 + c0}")
                    for w in range(cl):
                        wl = c0 + w
                        nc.tensor.matmul(
                            ps[:, w, :],
                            xb[:, wl, :],
                            wa[:, wl, :],
                            start=True,
                            stop=False,
                        )
                        nc.tensor.matmul(
                            ps[:, w, :],
                            x2[:, wl, :],
                            wb[:, wl, :],
                            start=False,
                            stop=True,
                        )
                    # alternate copy engine to spread the PSUM->SBUF casts
                    if ncopy % 2 == 0:
                        nc.scalar.copy(out=ob[:, c0 : c0 + cl, :], in_=ps[:])
                    else:
                        nc.vector.tensor_copy(out=ob[:, c0 : c0 + cl, :], in_=ps[:])
                    ncopy += 1
                nc.sync.dma_start(out=out_d[:, p0 : p0 + plen, :], in_=ob[:])

    nc.compile()
    return nc


def _get_nc():
    global _compiled_nc
    if _compiled_nc is None:
        _compiled_nc = _build_nc()
    return _compiled_nc


def shard_inputs(x, weights, bias):
    x = np.asarray(x, dtype=np.float32)
    weights = np.asarray(weights, dtype=np.float32)
    bias = np.asarray(bias, dtype=np.float32)

    xp = np.pad(x, ((0, 0), (0, 0), (1, 1)))
    # (IC, W+2, B) in bf16 once, host-side
    xpT = np.ascontiguousarray(xp.transpose(1, 2, 0)).astype(BF16)
    ones = np.ones((1, OWC, B), BF16)
    wT = weights.astype(BF16)
    bT = bias.astype(BF16)

    in_maps = []
    for c in range(NCORES):
        ws = c * OWC
        xb = np.concatenate(
            [xpT[:, ws : ws + OWC, :], xpT[:, ws + 1 : ws + 1 + OWC, :]], axis=0
        )
        x2 = np.concatenate([xpT[:, ws + 2 : ws + 2 + OWC, :], ones], axis=0)
        wsl = wT[ws : ws + OWC]  # (OWC, OC, IC, KS)
        wa = np.ascontiguousarray(wsl[:, :, :, 0:2].transpose(3, 2, 0, 1)).reshape(
            2 * IC, OWC, OC
        )
        wb = np.concatenate(
            [wsl[:, :, :, 2].transpose(2, 0, 1), bT[:, ws : ws + OWC].T[None]],
            axis=0,
        )
        in_maps.append(
            {
                "xb": np.ascontiguousarray(xb),
                "x2": np.ascontiguousarray(x2),
                "wa": np.ascontiguousarray(wa),
                "wb": np.ascontiguousarray(wb),
            }
        )
    return in_maps


def run_sharded(x, weights, bias, trace=False):
    nc = _get_nc()
    in_maps = shard_inputs(x, weights, bias)
    res = run_bass_kernel_spmd(nc, in_maps, list(range(NCORES)), trace=trace)
    out = np.empty((B, OC, W), np.float32)
    for c in range(NCORES):
        ws = c * OWC
        out[:, :, ws : ws + OWC] = (
            res.results[c]["out"].astype(np.float32).transpose(0, 2, 1)
        )
    return out, res


def kernel(x, weights, bias):
    out, _ = run_sharded(x, weights, bias)
    return out
